# revision 6
# baseline (speedup 1.0000x reference)
"""Trainium2 Bass kernel for nn_LogisticModel (logistic regression on
linear + pairwise-product + square features of x).

Math: for sample n,
  logit[n] = sum_i wlin[i] x[n,i]
           + sum_{i<j} Wp[k(i,j)] x[n,i] x[n,j]
           + sum_i Ws[i] x[n,i]^2 + b
           = x_n^T Q x_n + wlin . x_n + b
where Q[i,j] (i<j) = Wp at the row-major triu index, Q[i,i] = Ws[i],
Q lower triangle = 0.  So instead of materializing 131840 features per
sample we compute Y = X @ Q (a [128,512]@[512,512] matmul per core),
then z = rowsum(Y * X) + (X @ wlin + b), out = sigmoid(z).

Sharding: data-parallel over the batch dim across 8 cores (128 rows
each); Q / wlin / b are replicated (Q is 1 MB).  Host-side work is pure
layout: scatter W into Q, slice + transpose the x shards.

Layout note: each contraction chunk k ships as ONE DMA carrying
[Q rows 128k..128k+127 | xT rows 128k..128k+127] so the fp32 matmul has
a single semaphore wait (the fp32 LDWEIGHTS slot only holds one wait
command — walrus errors with "Too many sync wait commands" otherwise).
Same trick for the [wlin | b | ones] aux row.
"""

import sys

for _p in ("/opt/trn_rl_repo", "/root/.axon_site/_ro/trn_rl_repo"):
    if _p not in sys.path:
        sys.path.insert(0, _p)

import numpy as np

import concourse.bass as bass
import concourse.mybir as mybir
from concourse.tile import TileContext
from concourse.bass_utils import run_bass_kernel_spmd

DIM = 512
BATCH = 1024
N_CORES = 8
ROWS = BATCH // N_CORES  # 128 rows of x per core
KC = DIM // 128  # 4 contraction chunks
CW = DIM + ROWS  # combined chunk width: 512 Q cols + 128 xT cols

N_PAIRS = DIM * (DIM - 1) // 2  # 130816
_IU, _JU = np.triu_indices(DIM, k=1)  # row-major (i<j) order, matches reference

F32 = mybir.dt.float32


def _build_nc(mm_dtype=F32, use_ttr=False):
    nc = bass.Bass()

    x_d = nc.dram_tensor("x", [ROWS, DIM], F32, kind="ExternalInput")
    comb_d = nc.dram_tensor("comb", [KC, 128, CW], F32, kind="ExternalInput")
    aux_d = nc.dram_tensor("aux", [1, DIM + 1 + 128], F32, kind="ExternalInput")
    out_d = nc.dram_tensor("out", [ROWS, 1], F32, kind="ExternalOutput")

    with TileContext(nc) as tc:
        with (
            tc.tile_pool(name="sb", bufs=1) as sb,
            tc.tile_pool(name="ps", bufs=1, space="PSUM") as ps,
        ):
            # ---- loads ----
            x_sb = sb.tile([ROWS, DIM], F32)
            nc.sync.dma_start(x_sb[:, :], x_d[:, :])

            comb_sb = sb.tile([128, KC * CW], F32)
            for k in range(KC):
                nc.sync.dma_start(
                    comb_sb[:, k * CW : (k + 1) * CW], comb_d[k, :, :]
                )

            aux_sb = sb.tile([1, DIM + 1 + 128], F32)
            nc.sync.dma_start(aux_sb[:, :], aux_d[:, :])
            wlin_ap = aux_sb[:, 0:DIM]
            b_ap = aux_sb[:, DIM : DIM + 1]
            ones_ap = aux_sb[:, DIM + 1 : DIM + 1 + 128]

            # ---- PE: Y = X @ Q + ones^T wlin ; zb = b broadcast ----
            y_ps = ps.tile([128, DIM], F32)
            for k in range(KC):
                nc.tensor.matmul(
                    y_ps[:, :],
                    comb_sb[:, k * CW + DIM : (k + 1) * CW].bitcast(mm_dtype),
                    comb_sb[:, k * CW : k * CW + DIM].bitcast(mm_dtype),
                    start=(k == 0),
                    stop=False,
                )
            nc.tensor.matmul(y_ps[:, :], ones_ap, wlin_ap, start=False, stop=True)

            zb_ps = ps.tile([128, 1], F32)
            nc.tensor.matmul(zb_ps[:, :], ones_ap, b_ap, start=True, stop=True)

            # ---- DVE: z = rowsum(Y * X) + zb ; ACT: sigmoid ----
            prod_sb = sb.tile([128, DIM], F32)
            z_sb = sb.tile([128, 1], F32)
            o_sb = sb.tile([128, 1], F32)
            if use_ttr:
                nc.vector.tensor_tensor_reduce(
                    out=prod_sb[:, :],
                    in0=y_ps[:, :],
                    in1=x_sb[:, :],
                    scale=1.0,
                    scalar=zb_ps[:, :],
                    op0=mybir.AluOpType.mult,
                    op1=mybir.AluOpType.add,
                    accum_out=z_sb[:, :],
                )
                nc.scalar.activation(
                    o_sb[:, :], z_sb[:, :], mybir.ActivationFunctionType.Sigmoid
                )
            else:
                # this walrus build allows only ONE sync-wait per compute
                # instruction; sequence DVE ops so each needs at most one.
                dummy_sb = sb.tile([128, 1], F32)
                nc.vector.tensor_copy(dummy_sb[:, :], x_sb[:, 0:1])  # absorbs x-DMA wait
                nc.vector.tensor_mul(prod_sb[:, :], y_ps[:, :], x_sb[:, :])  # waits PE only
                nc.vector.reduce_sum(
                    z_sb[:, :], prod_sb[:, :], axis=mybir.AxisListType.X
                )
                zb_sb = sb.tile([128, 1], F32)
                nc.vector.tensor_copy(zb_sb[:, :], zb_ps[:, :])  # waits PE (zb mm)
                nc.scalar.activation(
                    o_sb[:, :],
                    z_sb[:, :],
                    mybir.ActivationFunctionType.Sigmoid,
                    bias=zb_sb[:, :],
                )

            nc.sync.dma_start(out_d[:, :], o_sb[:, :])

    _legalize_waits(nc)
    return nc


def _legalize_waits(nc, max_waits=1):
    """This walrus build rejects >1 sync-wait command per instruction
    ("Too many sync wait commands").  Hoist extra waits onto single-wait
    NoOps inserted just before the offending instruction (same engine,
    so ordering is preserved)."""

    def fix_block(bb):
        insts = getattr(bb, "instructions", None)
        if insts:
            new = []
            for inst in insts:
                si = getattr(inst, "sync_info", None)
                waits = list(si.on_wait) if si is not None and si.on_wait else []
                if len(waits) > max_waits:
                    keep = waits[-max_waits:]
                    for w in waits[:-max_waits]:
                        nop = mybir.InstNoOp(
                            name=f"waitfix-{nc.next_id()}",
                            engine=inst.engine,
                            ins=[],
                            outs=[],
                            sync_info=mybir.SyncInfo(on_wait=[w], on_update=[]),
                        )
                        new.append(nop)
                    si.on_wait = keep
                new.append(inst)
            bb.instructions = new
        for sub in getattr(bb, "blocks", None) or []:
            fix_block(sub)

    for blk in nc.m.functions[0].blocks:
        fix_block(blk)


def _prep_inputs(x, W, b):
    x = np.ascontiguousarray(np.asarray(x, dtype=np.float32))
    W = np.asarray(W, dtype=np.float32).reshape(-1)
    b = np.asarray(b, dtype=np.float32).reshape(())

    q = np.zeros((DIM, DIM), dtype=np.float32)
    q[_IU, _JU] = W[DIM : DIM + N_PAIRS]
    np.fill_diagonal(q, W[DIM + N_PAIRS :])

    aux = np.zeros((1, DIM + 1 + 128), dtype=np.float32)
    aux[0, :DIM] = W[:DIM]
    aux[0, DIM] = b
    aux[0, DIM + 1 :] = 1.0

    in_maps = []
    for c in range(N_CORES):
        xs = np.ascontiguousarray(x[c * ROWS : (c + 1) * ROWS])
        comb = np.empty((KC, 128, CW), dtype=np.float32)
        comb[:, :, :DIM] = q.reshape(KC, 128, DIM)
        comb[:, :, DIM:] = np.ascontiguousarray(xs.T).reshape(KC, 128, ROWS)
        in_maps.append({"x": xs, "comb": comb, "aux": aux})
    return in_maps


def _run(x, W, b, trace=False, mm_dtype=F32):
    nc = _build_nc(mm_dtype=mm_dtype)
    in_maps = _prep_inputs(x, W, b)
    res = run_bass_kernel_spmd(
        nc, in_maps, core_ids=list(range(N_CORES)), trace=trace
    )
    out = np.concatenate([r["out"] for r in res.results], axis=0)
    return out, res


def kernel(x, W, b):
    out, _ = _run(x, W, b)
    return out


# revision 10
# speedup vs baseline: 1.0984x; 1.0984x over previous
"""Trainium2 Bass kernel for nn_LogisticModel (logistic regression on
linear + pairwise-product + square features of x).

Math: for sample n,
  logit[n] = sum_i wlin[i] x[n,i]
           + sum_{i<j} Wp[k(i,j)] x[n,i] x[n,j]
           + sum_i Ws[i] x[n,i]^2 + b
           = x_n^T Q x_n + wlin . x_n + b
where Q[i,j] (i<j) = Wp at the row-major triu index, Q[i,i] = Ws[i],
Q lower triangle = 0.  So instead of materializing 131840 features per
sample we compute Y = X @ Q (a [128,512]@[512,512] matmul per core),
then z = rowsum(Y * X) + (X @ wlin + b), out = sigmoid(z).

Sharding: data-parallel over the batch dim across 8 cores (128 rows
each); Q / wlin / b are replicated (Q is 1 MB).  Host-side work is pure
layout: scatter W into Q, slice + transpose the x shards.

Layout note: each contraction chunk k ships as ONE DMA carrying
[Q rows 128k..128k+127 | xT rows 128k..128k+127] so the fp32 matmul has
a single semaphore wait (the fp32 LDWEIGHTS slot only holds one wait
command — walrus errors with "Too many sync wait commands" otherwise).
Same trick for the [wlin | b | ones] aux row.
"""

import sys

for _p in ("/opt/trn_rl_repo", "/root/.axon_site/_ro/trn_rl_repo"):
    if _p not in sys.path:
        sys.path.insert(0, _p)

import numpy as np

import concourse.bass as bass
import concourse.mybir as mybir
from concourse.tile import TileContext
from concourse.bass_utils import run_bass_kernel_spmd

DIM = 512
BATCH = 1024
N_CORES = 8
ROWS = BATCH // N_CORES  # 128 rows of x per core
KC = DIM // 128  # 4 contraction chunks
CW = DIM + ROWS  # combined chunk width: 512 Q cols + 128 xT cols

N_PAIRS = DIM * (DIM - 1) // 2  # 130816
_IU, _JU = np.triu_indices(DIM, k=1)  # row-major (i<j) order, matches reference

F32 = mybir.dt.float32


def _build_nc(mm_dtype=F32, use_ttr=False):
    nc = bass.Bass()

    x_d = nc.dram_tensor("x", [ROWS, DIM], F32, kind="ExternalInput")
    comb_d = nc.dram_tensor("comb", [KC, 128, CW], mm_dtype, kind="ExternalInput")
    aux_d = nc.dram_tensor("aux", [1, DIM + 1 + 128], F32, kind="ExternalInput")
    out_d = nc.dram_tensor("out", [ROWS, 1], F32, kind="ExternalOutput")

    with TileContext(nc) as tc:
        with (
            tc.tile_pool(name="sb", bufs=1) as sb,
            tc.tile_pool(name="ps", bufs=1, space="PSUM") as ps,
        ):
            # ---- loads ----
            # comb chunks first (they gate the matmuls), alternating the two
            # HWDGE dispatch engines (SP / ACT) — dispatch is ~0.7us each and
            # serializes per engine.  x is only needed by DVE much later.
            comb_sb = sb.tile([128, KC * CW], mm_dtype)
            for k in range(KC):
                eng = nc.sync if k % 2 == 0 else nc.scalar
                eng.dma_start(comb_sb[:, k * CW : (k + 1) * CW], comb_d[k, :, :])

            aux_sb = sb.tile([1, DIM + 1 + 128], F32)
            nc.scalar.dma_start(aux_sb[:, :], aux_d[:, :])

            x_sb = sb.tile([ROWS, DIM], F32)
            nc.sync.dma_start(x_sb[:, :], x_d[:, :])
            wlin_ap = aux_sb[:, 0:DIM]
            b_ap = aux_sb[:, DIM : DIM + 1]
            ones_ap = aux_sb[:, DIM + 1 : DIM + 1 + 128]

            # ---- PE: Y = X @ Q + ones^T wlin ; zb = b broadcast ----
            y_ps = ps.tile([128, DIM], F32)
            for k in range(KC):
                nc.tensor.matmul(
                    y_ps[:, :],
                    comb_sb[:, k * CW + DIM : (k + 1) * CW],
                    comb_sb[:, k * CW : k * CW + DIM],
                    start=(k == 0),
                    stop=False,
                )
            nc.tensor.matmul(y_ps[:, :], ones_ap, wlin_ap, start=False, stop=True)

            zb_ps = ps.tile([128, 1], F32)
            nc.tensor.matmul(zb_ps[:, :], ones_ap, b_ap, start=True, stop=True)

            # ---- DVE: z = rowsum(Y * X) + zb ; ACT: sigmoid ----
            prod_sb = sb.tile([128, DIM], F32)
            z_sb = sb.tile([128, 1], F32)
            o_sb = sb.tile([128, 1], F32)
            if use_ttr:
                nc.vector.tensor_tensor_reduce(
                    out=prod_sb[:, :],
                    in0=y_ps[:, :],
                    in1=x_sb[:, :],
                    scale=1.0,
                    scalar=zb_ps[:, :],
                    op0=mybir.AluOpType.mult,
                    op1=mybir.AluOpType.add,
                    accum_out=z_sb[:, :],
                )
                nc.scalar.activation(
                    o_sb[:, :], z_sb[:, :], mybir.ActivationFunctionType.Sigmoid
                )
            else:
                # this walrus build allows only ONE sync-wait per compute
                # instruction; sequence DVE ops so each needs at most one.
                dummy_sb = sb.tile([128, 1], F32)
                nc.vector.tensor_copy(dummy_sb[:, :], x_sb[:, 0:1])  # absorbs x-DMA wait
                nc.vector.tensor_mul(prod_sb[:, :], y_ps[:, :], x_sb[:, :])  # waits PE only
                nc.vector.reduce_sum(
                    z_sb[:, :], prod_sb[:, :], axis=mybir.AxisListType.X
                )
                zb_sb = sb.tile([128, 1], F32)
                nc.vector.tensor_copy(zb_sb[:, :], zb_ps[:, :])  # waits PE (zb mm)
                nc.scalar.activation(
                    o_sb[:, :],
                    z_sb[:, :],
                    mybir.ActivationFunctionType.Sigmoid,
                    bias=zb_sb[:, :],
                )

            nc.sync.dma_start(out_d[:, :], o_sb[:, :])

    _legalize_waits(nc)
    return nc


def _legalize_waits(nc, max_waits=1):
    """This walrus build rejects >1 sync-wait command per instruction
    ("Too many sync wait commands").  Hoist extra waits onto single-wait
    NoOps inserted just before the offending instruction (same engine,
    so ordering is preserved)."""

    def fix_block(bb):
        insts = getattr(bb, "instructions", None)
        if insts:
            new = []
            for inst in insts:
                si = getattr(inst, "sync_info", None)
                waits = list(si.on_wait) if si is not None and si.on_wait else []
                if len(waits) > max_waits:
                    keep = waits[-max_waits:]
                    for w in waits[:-max_waits]:
                        nop = mybir.InstNoOp(
                            name=f"waitfix-{nc.next_id()}",
                            engine=inst.engine,
                            ins=[],
                            outs=[],
                            sync_info=mybir.SyncInfo(on_wait=[w], on_update=[]),
                        )
                        new.append(nop)
                    si.on_wait = keep
                new.append(inst)
            bb.instructions = new
        for sub in getattr(bb, "blocks", None) or []:
            fix_block(sub)

    for blk in nc.m.functions[0].blocks:
        fix_block(blk)


def _prep_inputs(x, W, b):
    x = np.ascontiguousarray(np.asarray(x, dtype=np.float32))
    W = np.asarray(W, dtype=np.float32).reshape(-1)
    b = np.asarray(b, dtype=np.float32).reshape(())

    q = np.zeros((DIM, DIM), dtype=np.float32)
    q[_IU, _JU] = W[DIM : DIM + N_PAIRS]
    np.fill_diagonal(q, W[DIM + N_PAIRS :])

    aux = np.zeros((1, DIM + 1 + 128), dtype=np.float32)
    aux[0, :DIM] = W[:DIM]
    aux[0, DIM] = b
    aux[0, DIM + 1 :] = 1.0

    in_maps = []
    for c in range(N_CORES):
        xs = np.ascontiguousarray(x[c * ROWS : (c + 1) * ROWS])
        comb = np.empty((KC, 128, CW), dtype=np.float32)
        comb[:, :, :DIM] = q.reshape(KC, 128, DIM)
        comb[:, :, DIM:] = np.ascontiguousarray(xs.T).reshape(KC, 128, ROWS)
        in_maps.append({"x": xs, "comb": comb, "aux": aux})
    return in_maps


def _run(x, W, b, trace=False, mm_dtype=F32):
    nc = _build_nc(mm_dtype=mm_dtype)
    in_maps = _prep_inputs(x, W, b)
    res = run_bass_kernel_spmd(
        nc, in_maps, core_ids=list(range(N_CORES)), trace=trace
    )
    out = np.concatenate([r["out"] for r in res.results], axis=0)
    return out, res


def kernel(x, W, b):
    out, _ = _run(x, W, b)
    return out
